# revision 32
# baseline (speedup 1.0000x reference)
"""ChebConv (K=4) GNN layer on 8 Trainium2 NeuronCores — v4.

Trace findings vs v2 baseline (2.43ms): kernel is DMA-ENGINE-bound.
Gathers are 256B/edge descriptors at ~21.2ns/engine (HBM 512B min
burst, so bytes below 512 are free but descriptor count is not); the
one-hot scatter (S_w) stream was as large as the gathers.

Changes vs v2 (2.43ms -> ~1.69ms):
  - Edges fully sorted by dst within each (quarter, group); caps at
    (v,g) granularity only -> gather padding 11.9% -> 5.9%.
  - VARIABLE-WIDTH scatter units: one unit per 128-edge block, matmul
    moving width = cross-core dst span (mean ~51) at a data-driven
    base -> S_w stream 57.3 -> 21.6 MB/step, and fewer PE instructions.
    (PSUM has per-f32-element has_written bits, so overlapping
    accumulation windows are safe on HW; CoreSim's pending-zero model
    is stricter -> KLIB_ALIGN_BASES=1 for sim runs.)
  - Output written feature-major-tiled ([128, ntiles*F], one descriptor
    per partition instead of 256B per node row); host detiles.
  - s==1 full-width zero-start matmul kept (guards unit-less columns).

Measured dead ends: >1024-idx gather calls crash NRT in single-packet
mode; multi-packet mode works but is slower (packetization amortizes
the sub-512B fabric overhead); greedy/finer quarter layouts (AllGather
rebalancing) were both slower than 4 balanced quarters.
"""

import os
import sys
import types

if "/opt/trn_rl_repo" not in sys.path:
    sys.path.insert(0, "/opt/trn_rl_repo")

import numpy as np


def _install_ntff_hook():
    if "antenv.axon_hooks" in sys.modules:
        return
    try:
        import antenv
    except ImportError:
        return
    mod = types.ModuleType("antenv.axon_hooks")
    state = {"hook": None}
    mod.set_axon_ntff_profile_hook = lambda h: state.__setitem__("hook", h)
    mod.get_axon_ntff_profile_hook = lambda: state["hook"]
    sys.modules["antenv.axon_hooks"] = mod
    antenv.axon_hooks = mod
    try:
        from trn_agent_boot.trn_boot import _ntff_profile_via_ctypes

        mod.set_axon_ntff_profile_hook(
            _ntff_profile_via_ctypes("/opt/axon/libaxon_pjrt.so")
        )
    except Exception:
        pass


F = 128
GROUP = 512   # dst nodes per PSUM accumulation group (one f32 bank)
SW = int(os.environ.get("KLIB_SW", "64"))  # scatter window width
NB = int(os.environ.get("KLIB_NB", "2"))   # dst groups per band
NQ = 4        # SWDGE queues
MAXWIN = int(os.environ.get("KLIB_MAXWIN", "32768"))  # int16 index reach


class Plan:
    __slots__ = (
        "cores", "n", "nshard", "k", "ngroups", "gwidths", "ntiles", "npad",
        "nquart", "qbounds", "qlens", "qsizes", "qof_g",
        "bands", "runs", "calls", "total_units", "idx_cols",
        "max_call_blocks", "band_u0", "band_ucnt", "max_band_sw",
        "uwidth", "uoff", "fold_dve",
        "idx", "wcol1", "wcol2", "dstl", "xq", "xt", "call", "weight",
    )


def _quarter_layout(ngroups, gwidths, cores):
    """Split dst groups into balanced windows <= MAXWIN rows each.

    More/smaller windows than the int16 reach requires: the AllGathers
    serialize on the CC cores, so finer windows pipeline better and the
    step-gating final AllGather is shorter.
    """
    cap = MAXWIN // cores
    nq = int(os.environ.get("KLIB_NQUART", "4"))
    while True:
        per = (ngroups + nq - 1) // nq
        bounds = []
        g0 = 0
        while g0 < ngroups:
            g1 = min(g0 + per, ngroups)
            bounds.append((g0, g1))
            g0 = g1
        if all(sum(gwidths[a:b]) <= cap for a, b in bounds):
            return bounds
        nq += 1
        assert nq <= ngroups, (ngroups, cap)


def _pack(x, filter_coeff, weight, edge_w, src, dst, n, cores, k):
    p = Plan()
    p.cores, p.n, p.k = cores, n, k
    nshard = n // cores
    assert n % cores == 0
    p.nshard = nshard
    ngroups = (nshard + GROUP - 1) // GROUP
    p.ngroups = ngroups
    p.gwidths = [min(GROUP, nshard - g * GROUP) for g in range(ngroups)]
    p.ntiles = (nshard + 127) // 128
    p.npad = p.ntiles * 128

    p.qbounds = _quarter_layout(ngroups, p.gwidths, cores)
    p.nquart = len(p.qbounds)
    p.qlens = [sum(p.gwidths[a:b]) for a, b in p.qbounds]
    p.qsizes = [q * cores for q in p.qlens]
    nwin = p.nquart
    # group -> (quarter, is_last_of_quarter, row offset within quarter)
    p.qof_g = {}
    g2q = np.zeros(ngroups, dtype=np.int64)
    g2qoff = np.zeros(ngroups, dtype=np.int64)
    for qi, (ga, gb) in enumerate(p.qbounds):
        off = 0
        for g in range(ga, gb):
            p.qof_g[g] = (qi, g == gb - 1, off)
            g2q[g] = qi
            g2qoff[g] = off
            off += p.gwidths[g]

    src = np.asarray(src)
    dst = np.asarray(dst)
    edge_w = np.asarray(edge_w, dtype=np.float32)
    # Every dst column with >=1 edge is covered by some unit window, so
    # PSUM could start on the first scatter matmul with the T_{s-2}
    # subtract folded into a DVE copy (requires min in-degree >= 1;
    # degree-0 dst columns would read stale PSUM). Measured neutral on
    # HW (1711us vs 1690/1701us) - off by default, KLIB_FOLD_DVE=1 to
    # re-enable.
    p.fold_dve = bool(
        os.environ.get("KLIB_FOLD_DVE")
        and np.bincount(dst, minlength=n).min() >= 1
    )

    # --- src relabel into quarter space ---
    sc_owner = src // nshard
    sc_r = src - sc_owner * nshard
    sc_g = np.minimum(sc_r // GROUP, ngroups - 1)
    qlens_arr = np.array(p.qlens, dtype=np.int64)
    v_of = g2q[sc_g]
    off_of = (sc_owner * qlens_arr[v_of] + g2qoff[sc_g]
              + (sc_r - sc_g * GROUP))
    assert off_of.max() < MAXWIN

    # --- dst bucketing: per (v, g), fully sorted by dst-within-group ---
    owner = dst // nshard
    dloc = dst - owner * nshard
    g_of = dloc // GROUP
    dwg = dloc - g_of * GROUP  # dst within group

    key = (v_of * ngroups + g_of).astype(np.int64)
    nbuck = nwin * ngroups
    counts = np.zeros((cores, nbuck), dtype=np.int64)
    percore = []   # per core: (src_off, dwg, w) sorted by (bucket, dwg)
    for c in range(cores):
        m = owner == c
        kc = key[m]
        order = np.lexsort((dwg[m], kc))
        percore.append((off_of[m][order], dwg[m][order], edge_w[m][order],
                        g_of[m][order]))
        counts[c] = np.bincount(kc, minlength=nbuck)

    caps = counts.max(axis=0).reshape(nwin, ngroups)
    cstarts = [np.concatenate([[0], np.cumsum(counts[c])])
               for c in range(cores)]

    # per-(v,g) per-core sorted dwg segment views for span computation
    def seg(c, v, g):
        b = v * ngroups + g
        s0, s1 = cstarts[c][b], cstarts[c][b + 1]
        return s0, s1

    # --- static runs (v, g) + merged gather calls per (band, v) ---
    nbands = (ngroups + NB - 1) // NB
    p.bands = [(b * NB, min(b * NB + NB, ngroups)) for b in range(nbands)]

    runs = {}
    total_units = 0
    idx_cols = 0
    calls = []
    max_call_blocks = 0
    for b, (ga, gb) in enumerate(p.bands):
        for v in range(nwin):
            call_runs = []
            call_C = 0
            for g in range(ga, gb):
                tot = int(caps[v, g])
                if tot == 0:
                    continue
                C = (tot + 127) // 128 * 128
                units = []
                for s in range(C // 128):
                    lo, hi = None, None
                    for c in range(cores):
                        s0, s1 = seg(c, v, g)
                        cnt = s1 - s0
                        blo, bhi = 128 * s, min(128 * s + 128, cnt)
                        if blo >= bhi:
                            continue
                        dseg = percore[c][1][s0 + blo: s0 + bhi]
                        dl, dh = int(dseg[0]), int(dseg[-1])
                        lo = dl if lo is None else min(lo, dl)
                        hi = dh if hi is None else max(hi, dh)
                    if lo is None:
                        continue
                    if os.environ.get("KLIB_ALIGN_BASES"):
                        # SW-aligned slots: CoreSim's pending-zero model needs
                        # window-disjoint PSUM writes (HW does not: per-element
                        # has_written bits)
                        for j in range(lo // SW, hi // SW + 1):
                            units.append(
                                (s, SW * j, SW, total_units + len(units))
                            )
                    else:
                        # one variable-width unit per block: matmul moving
                        # width = cross-core dst span (rounded to even)
                        width = min((hi - lo + 2) // 2 * 2, GROUP - lo)
                        units.append(
                            (s, lo, width, total_units + len(units))
                        )
                r = dict(v=v, g=g, C=C, units=units,
                         u0=total_units, blk0=call_C // 128)
                runs[(v, g)] = r
                call_runs.append(r)
                total_units += len(units)
                call_C += C
            if call_C == 0:
                continue
            calls.append(dict(band=b, v=v, idx_off=idx_cols, C=call_C,
                              runs=call_runs))
            idx_cols += call_C // 16
            max_call_blocks = max(max_call_blocks, call_C // 128)
    p.runs = runs
    p.calls = calls
    # per-unit S_w column widths -> cumulative offsets
    uwidth = np.zeros(max(total_units, 1), dtype=np.int64)
    for r in runs.values():
        for (_s, _b, w, ucol) in r["units"]:
            uwidth[ucol] = w
    p.uwidth = uwidth
    p.uoff = np.concatenate([[0], np.cumsum(uwidth)])
    p.band_u0 = {}
    p.band_ucnt = {}
    for b in range(nbands):
        us = [u[3] for (vv, gg), r in runs.items()
              if p.bands[b][0] <= gg < p.bands[b][1] for u in r["units"]]
        p.band_u0[b] = min(us) if us else 0
        p.band_ucnt[b] = len(us)
        if us:
            assert max(us) - min(us) + 1 == len(us), f"band {b} units gap"
    p.max_band_sw = max(
        max((int(p.uoff[p.band_u0[b] + p.band_ucnt[b]]
             - p.uoff[p.band_u0[b]]) for b in range(nbands)), default=SW),
        SW,
    )
    p.total_units = max(total_units, 1)
    p.idx_cols = max(idx_cols, 16)
    p.max_call_blocks = max(max_call_blocks, 1)

    idx_all = np.zeros((cores, 128, p.idx_cols), dtype=np.int16)
    wcol1 = np.zeros((cores, 128, p.total_units), dtype=np.float32)
    dstl = np.full((cores, 128, p.total_units), -1.0, dtype=np.float32)

    for c in range(cores):
        sc, dc, wc, _ = percore[c]
        for call in calls:
            for r in call["runs"]:
                v, g = r["v"], r["g"]
                C = r["C"]
                s0, s1 = seg(c, v, g)
                cnt = s1 - s0
                buf_src = np.zeros(C, dtype=np.int64)
                buf_dl = np.full(C, -1.0, dtype=np.float32)
                buf_w = np.zeros(C, dtype=np.float32)
                buf_src[:cnt] = sc[s0:s1]
                buf_dl[:cnt] = dc[s0:s1]
                buf_w[:cnt] = wc[s0:s1]
                blk = buf_src.reshape(C // 16, 16).T.astype(np.int16)
                o = call["idx_off"] + r["blk0"] * 8
                idx_all[c, :, o: o + C // 16] = np.tile(blk, (8, 1))
                covered = np.zeros(C, dtype=bool)
                for (s, base, width, ucol) in r["units"]:
                    sl = slice(128 * s, 128 * s + 128)
                    seg_dl = buf_dl[sl]
                    seg_w = buf_w[sl]
                    rel = seg_dl - base
                    inw = ((seg_dl >= 0) & (rel >= 0) & (rel < width)
                           & ~covered[sl])
                    dstl[c, :, ucol] = np.where(inw, rel, -1.0)
                    wcol1[c, :, ucol] = np.where(inw, seg_w, 0.0)
                    covered[sl] |= inw
                miss = (buf_dl >= 0) & ~covered
                assert not miss.any()

    p.idx = idx_all
    p.dstl = dstl
    p.wcol1 = wcol1
    p.wcol2 = 2.0 * wcol1

    x = np.asarray(x, dtype=np.float32)
    fc = np.asarray(filter_coeff, dtype=np.float32)

    # x relabeled into quarter spaces (replicated inputs)
    xq = []
    for qi, (ga, gb) in enumerate(p.qbounds):
        ra = ga * GROUP
        rb = ra + p.qlens[qi]
        xq.append(np.concatenate(
            [x[c * nshard + ra: c * nshard + rb] for c in range(cores)],
            axis=0,
        ))
    p.xq = xq

    p.xt = np.stack([
        np.pad(x[c * nshard:(c + 1) * nshard].T,
               ((0, 0), (0, p.npad - nshard)))
        for c in range(cores)
    ])

    call_c = np.zeros((cores, 128, k * p.ntiles), dtype=np.float32)
    for c in range(cores):
        cc = np.zeros((k, p.npad), dtype=np.float32)
        cc[:, :nshard] = fc[:, c * nshard:(c + 1) * nshard]
        call_c[c] = (
            cc.reshape(k, p.ntiles, 128).transpose(2, 0, 1).reshape(128, -1)
        )
    p.call = call_c
    p.weight = np.ascontiguousarray(np.asarray(weight, dtype=np.float32))
    return p


def _build(p):
    import concourse.bacc as bacc
    import concourse.mybir as mybir
    import concourse.tile as tile

    dt = mybir.dt
    f32 = dt.float32
    bf16 = dt.bfloat16
    k = p.k
    ntiles, ngroups = p.ntiles, p.ngroups
    npad = p.npad
    nwin = p.nquart

    max_step = int(os.environ.get("KLIB_MAX_STEP", str(k - 1)))
    no_ag = bool(os.environ.get("KLIB_NO_AG"))
    # >1024 idxs per dma_gather crashes NRT with single_packet=True (packet
    # limit: 16 engines x 64 descs/packet); multi-packet mode lifts it
    callcap = int(os.environ.get("KLIB_CALLCAP", "1024"))
    single_packet = os.environ.get("KLIB_SINGLE_PACKET", "1") != "0"

    nc = bacc.Bacc(None, target_bir_lowering=False, debug=False,
                   num_devices=p.cores, num_swdge_queues=NQ)

    xq_d = [nc.dram_tensor(f"xq{q}", [p.qsizes[q], F], bf16,
                           kind="ExternalInput") for q in range(nwin)]
    xt_d = nc.dram_tensor("xt", [F, npad], bf16, kind="ExternalInput")
    w_d = nc.dram_tensor("weight", [k, F, F], bf16, kind="ExternalInput")
    call_d = nc.dram_tensor("call", [128, k * ntiles], f32,
                            kind="ExternalInput")
    idx_d = nc.dram_tensor("idx", [128, p.idx_cols], dt.int16,
                           kind="ExternalInput")
    sw_cols = int(p.uoff[-1]) if p.uoff[-1] > 0 else SW
    sw1_d = nc.dram_tensor("sw1", [128, sw_cols], bf16,
                           kind="ExternalInput")
    sw2_d = nc.dram_tensor("sw2", [128, sw_cols], bf16,
                           kind="ExternalInput")
    ident_d = nc.dram_tensor("ident", [128, 128], bf16,
                             kind="ExternalInput")
    negi_d = nc.dram_tensor("negi", [128, 128], bf16, kind="ExternalInput")
    identf_d = nc.dram_tensor("identf", [128, 128], f32,
                              kind="ExternalInput")
    # feature-major-tiled output: out_d[p, t*F + f] = out[t*128 + p, f]
    out_d = nc.dram_tensor("out", [128, ntiles * F], bf16,
                           kind="ExternalOutput")

    tq = {}
    tfq = {}
    for s in range(1, k - 1):
        for q in range(nwin):
            tq[(s, q)] = nc.dram_tensor(f"t{s}q{q}", [p.qlens[q], F], bf16)
            tfq[(s, q)] = nc.dram_tensor(f"t{s}f{q}", [p.qsizes[q], F],
                                         bf16, addr_space="Shared")
    t1p_d = nc.dram_tensor("t1p", [F, npad], bf16)

    with tile.TileContext(nc) as tc:
        with (
            tc.tile_pool(name="const", bufs=1) as constp,
            tc.tile_pool(name="idxp", bufs=2) as idxp,
            tc.tile_pool(name="stage", bufs=2) as stagep,
            tc.tile_pool(name="swp", bufs=2) as swp,
            tc.tile_pool(name="work", bufs=2) as workp,
            tc.tile_pool(name="acc", bufs=1) as accp,
            tc.tile_pool(name="psU", bufs=2, space="PSUM") as psup,
            tc.tile_pool(name="psY", bufs=2, space="PSUM") as psyp,
            tc.tile_pool(name="psT", bufs=2, space="PSUM") as pstp,
        ):
            ident_t = constp.tile([128, 128], bf16)
            negi_t = constp.tile([128, 128], bf16)
            identf_t = constp.tile([128, 128], f32)
            zeros_t = constp.tile([128, GROUP], bf16)
            wk_t = constp.tile([128, k * 128], bf16)
            call_t = constp.tile([128, k * ntiles], f32)
            out_acc = accp.tile([128, npad], bf16)

            nc.sync.dma_start(ident_t[:], ident_d[:])
            nc.sync.dma_start(negi_t[:], negi_d[:])
            nc.sync.dma_start(identf_t[:], identf_d[:])
            for s in range(k):
                nc.sync.dma_start(
                    wk_t[:, s * 128:(s + 1) * 128], w_d[s, :, :]
                )
            nc.sync.dma_start(call_t[:], call_d[:])
            nc.gpsimd.memset(zeros_t[:], 0.0)
            nc.vector.memset(out_acc[:], 0.0)

            def out_update(step, g, gw, src_psum):
                for i in range((gw + 127) // 128):
                    wi = min(128, gw - 128 * i)
                    t_idx = g * (GROUP // 128) + i
                    ccol = call_t[:wi, step * ntiles + t_idx
                                  : step * ntiles + t_idx + 1]
                    nc.vector.scalar_tensor_tensor(
                        out_acc[:wi, t_idx * 128:(t_idx + 1) * 128],
                        src_psum[:wi, i * 128: i * 128 + 128],
                        ccol,
                        out_acc[:wi, t_idx * 128:(t_idx + 1) * 128],
                        mybir.AluOpType.mult,
                        mybir.AluOpType.add,
                    )

            # ---- step 0: out += c0 * (x @ W0) ----
            for g in range(ngroups):
                gw = p.gwidths[g]
                xt_tile = workp.tile([128, GROUP], bf16, tag="xt")
                nc.sync.dma_start(
                    xt_tile[:, :gw], xt_d[:, g * GROUP: g * GROUP + gw]
                )
                psY = psyp.tile([128, GROUP], f32)
                nc.tensor.matmul(
                    psY[:, :gw], wk_t[:, 0:128], xt_tile[:, :gw],
                    start=True, stop=True,
                )
                ys = workp.tile([128, GROUP], f32, tag="ys")
                nc.scalar.copy(ys[:, :gw], psY[:, :gw])
                psT = pstp.tile([128, GROUP], f32)
                for i in range((gw + 127) // 128):
                    wi = min(128, gw - 128 * i)
                    nc.tensor.transpose(
                        psT[:wi, i * 128: i * 128 + 128],
                        ys[:, i * 128: i * 128 + wi],
                        identf_t[:],
                    )
                out_update(0, g, gw, psT)

            # ---- steps 1..k-1 ----
            calls_by_band = {}
            for call in p.calls:
                calls_by_band.setdefault(call["band"], []).append(call)
            qrr = [0]

            def issue_band_gathers(s, b):
                out = {}
                if b in calls_by_band and p.band_ucnt.get(b, 0):
                    u0 = p.band_u0[b]
                    un = p.band_ucnt[b]
                    o0 = int(p.uoff[u0])
                    o1 = int(p.uoff[u0 + un])
                    sw_d = sw1_d if s == 1 else sw2_d
                    swt = swp.tile([128, p.max_band_sw], bf16, tag="sw")
                    nc.sync.dma_start(
                        swt[:, : o1 - o0], sw_d[:, o0: o1]
                    )
                    out["sw"] = (swt, o0)
                for call in calls_by_band.get(b, []):
                    v = call["v"]
                    C = call["C"]
                    src_full = xq_d[v] if s == 1 else tfq[(s - 1, v)]
                    it = idxp.tile([128, p.max_call_blocks * 8], dt.int16,
                                   tag=f"idx{v}")
                    nc.sync.dma_start(
                        it[:, : C // 16],
                        idx_d[:, call["idx_off"]: call["idx_off"] + C // 16],
                    )
                    st = stagep.tile([128, p.max_call_blocks, F], bf16,
                                     tag=f"st{v}")
                    nsplit = -(-C // callcap)
                    csz = -(-(C // 128) // nsplit) * 128
                    for q0 in range(0, C, csz):
                        cl = min(csz, C - q0)
                        nc.gpsimd.dma_gather(
                            st[:, q0 // 128: (q0 + cl) // 128, :],
                            src_full[:, :],
                            it[:, q0 // 16: (q0 + cl) // 16],
                            cl, cl, F,
                            single_packet=single_packet,
                            queue_num=qrr[0] % NQ,
                        )
                        qrr[0] += 1
                    out[v] = st
                return out

            for s in range(1, min(k, max_step + 1)):
                stage_tiles = issue_band_gathers(s, 0)
                for b, (ga, gb) in enumerate(p.bands):
                    next_tiles = (issue_band_gathers(s, b + 1)
                                  if b + 1 < len(p.bands) else {})
                    for g in range(ga, gb):
                        gw = p.gwidths[g]
                        runs_g = [p.runs[(v, g)] for v in range(nwin)
                                  if (v, g) in p.runs]
                        n_units = sum(len(r["units"]) for r in runs_g)
                        psU = psup.tile([128, GROUP], f32)
                        km2 = None
                        if s >= 2:
                            km2 = workp.tile([128, GROUP], bf16, tag="km2")
                            km2_src = xt_d if s == 2 else t1p_d
                            nc.sync.dma_start(
                                km2[:, :gw],
                                km2_src[:, g * GROUP: g * GROUP + gw],
                            )
                        if p.fold_dve and n_units > 0:
                            # units start PSUM themselves; T_{s-2} subtract
                            # folds into the DVE copy below (decouples the
                            # unit chain from the km2 load)
                            started = False
                        elif s >= 2:
                            if gw < GROUP:
                                nc.vector.memset(km2[:, gw:], 0.0)
                            nc.tensor.matmul(
                                psU[:], negi_t[:], km2[:],
                                start=True, stop=(n_units == 0),
                                skip_group_check=True,
                            )
                            started = True
                        else:
                            # full-width zero start: every PSUM byte written
                            # (avoids stale reads from unit-less slots)
                            nc.tensor.matmul(
                                psU[:], zeros_t[:, :128], zeros_t[:],
                                start=True, stop=(n_units == 0),
                                skip_group_check=True,
                            )
                            started = True
                        last_u = max(
                            (u[3] for r in runs_g for u in r["units"]),
                            default=None,
                        )
                        swt, sw_o0 = stage_tiles["sw"]
                        for r in runs_g:
                            st = stage_tiles[r["v"]]
                            for (sl, base, width, ucol) in r["units"]:
                                uo = int(p.uoff[ucol]) - sw_o0
                                nc.tensor.matmul(
                                    psU[:, base: base + width],
                                    st[:, r["blk0"] + sl, :],
                                    swt[:, uo: uo + width],
                                    start=(not started),
                                    stop=(ucol == last_u),
                                    skip_group_check=True,
                                )
                                started = True
                        # T'^T feature-major bf16
                        ts = workp.tile([128, GROUP], bf16, tag="ts")
                        if s >= 2 and p.fold_dve and n_units > 0:
                            # ts = psU - T_{s-2} (recurrence folded on DVE)
                            nc.vector.scalar_tensor_tensor(
                                ts[:, :gw], psU[:, :gw], 1.0, km2[:, :gw],
                                mybir.AluOpType.mult,
                                mybir.AluOpType.subtract,
                            )
                        else:
                            nc.scalar.copy(ts[:, :gw], psU[:, :gw])
                        if s == 1:
                            nc.sync.dma_start(
                                t1p_d[:, g * GROUP: g * GROUP + gw],
                                ts[:, :gw],
                            )
                        qi, qlast, qoff = p.qof_g[g]
                        if s <= k - 2:
                            psN = pstp.tile([128, GROUP], bf16, tag="psN",
                                            padded_shape=[128, 2 * GROUP])
                            for i in range((gw + 127) // 128):
                                wi = min(128, gw - 128 * i)
                                nc.tensor.transpose(
                                    psN[:wi, i * 128: i * 128 + 128],
                                    ts[:, i * 128: i * 128 + wi],
                                    ident_t[:],
                                )
                            tn = workp.tile([128, GROUP], bf16, tag="tn")
                            for i in range((gw + 127) // 128):
                                wi = min(128, gw - 128 * i)
                                nc.scalar.copy(
                                    tn[:wi, i * 128: i * 128 + 128],
                                    psN[:wi, i * 128: i * 128 + 128],
                                )
                                nc.sync.dma_start(
                                    tq[(s, qi)].ap()[
                                        qoff + i * 128: qoff + i * 128 + wi,
                                        :,
                                    ],
                                    tn[:wi, i * 128: i * 128 + 128],
                                )
                        # Y^T = W_s^T @ T'^T ; out += c_s * Y
                        psY = psyp.tile([128, GROUP], f32)
                        nc.tensor.matmul(
                            psY[:, :gw], wk_t[:, s * 128: s * 128 + 128],
                            ts[:, :gw], start=True, stop=True,
                        )
                        ys = workp.tile([128, GROUP], f32, tag="ys")
                        nc.scalar.copy(ys[:, :gw], psY[:, :gw])
                        psT = pstp.tile([128, GROUP], f32)
                        for i in range((gw + 127) // 128):
                            wi = min(128, gw - 128 * i)
                            nc.tensor.transpose(
                                psT[:wi, i * 128: i * 128 + 128],
                                ys[:, i * 128: i * 128 + wi],
                                identf_t[:],
                            )
                        out_update(s, g, gw, psT)
                        if qlast and s <= k - 2 and not no_ag:
                            nc.gpsimd.collective_compute(
                                "AllGather",
                                mybir.AluOpType.bypass,
                                replica_groups=[list(range(p.cores))],
                                ins=[tq[(s, qi)].ap().opt()],
                                outs=[tfq[(s, qi)].ap().opt()],
                            )
                    stage_tiles = next_tiles

            nc.sync.dma_start(out_d[:, :], out_acc[:])

    nc.compile()
    return nc


def _bf16(a):
    import ml_dtypes

    return np.ascontiguousarray(
        np.asarray(a, dtype=np.float32).astype(ml_dtypes.bfloat16)
    )


def _sw_expand(dstl, wcol, uwidth, uoff):
    import ml_dtypes

    total = int(uoff[-1])
    if total == 0:
        return np.zeros((128, SW), dtype=ml_dtypes.bfloat16)
    col_unit = np.repeat(np.arange(len(uwidth)), uwidth)
    col_rel = (np.arange(total) - uoff[col_unit]).astype(np.float32)
    out = np.empty((128, total), dtype=ml_dtypes.bfloat16)
    CH = 65536
    for j0 in range(0, total, CH):
        j1 = min(j0 + CH, total)
        cu = col_unit[j0:j1]
        out[:, j0:j1] = ((dstl[:, cu] == col_rel[None, j0:j1])
                         * wcol[:, cu].astype(np.float32))
    return out


def _make_in_maps(p):
    ident = np.eye(128, dtype=np.float32)
    maps = []
    for c in range(p.cores):
        m = {
            "xt": _bf16(p.xt[c]),
            "weight": _bf16(p.weight),
            "call": p.call[c],
            "idx": p.idx[c],
            "sw1": _sw_expand(p.dstl[c], p.wcol1[c], p.uwidth, p.uoff),
            "sw2": _sw_expand(p.dstl[c], p.wcol2[c], p.uwidth, p.uoff),
            "ident": _bf16(ident),
            "negi": _bf16(-ident),
            "identf": ident,
        }
        for q in range(p.nquart):
            m[f"xq{q}"] = _bf16(p.xq[q])
        maps.append(m)
    return maps


_LAST_EXEC_NS = None


def run(x, filter_coeff, weight, bias, edge_w, src, dst, *, cores=8,
        trace=False, sim=False):
    global _LAST_EXEC_NS
    n, f = np.asarray(x).shape
    assert f == F
    k = np.asarray(weight).shape[0]
    p = _pack(x, filter_coeff, weight, edge_w, src, dst, n, cores, k)
    nc = _build(p)
    in_maps = _make_in_maps(p)

    if sim:
        from concourse.bass_interp import MultiCoreSim

        msim = MultiCoreSim(nc, cores)
        for c in range(cores):
            for name, arr in in_maps[c].items():
                msim.cores[c].tensor(name)[:] = arr
        msim.simulate()
        outs = [
            np.array(msim.cores[c].mem_tensor("out")) for c in range(cores)
        ]
    else:
        _install_ntff_hook()
        from concourse import bass_utils

        res = bass_utils.run_bass_kernel_spmd(
            nc, in_maps, core_ids=list(range(cores)), trace=trace
        )
        _LAST_EXEC_NS = res.exec_time_ns
        outs = [res.results[c]["out"] for c in range(cores)]

    nshard = n // cores
    ntiles = p.ntiles
    shards = []
    for o in outs:
        a = np.asarray(o, dtype=np.float32).reshape(128, ntiles, F)
        a = a.transpose(1, 0, 2).reshape(ntiles * 128, F)
        shards.append(a[:nshard])
    full = np.concatenate(shards, axis=0)
    return (full + np.asarray(bias, dtype=np.float32)[None, :]).astype(
        np.float32
    )


def kernel(x, filter_coeff, weight, bias, edge_w, src, dst):
    trace = bool(os.environ.get("KBENCH_TRACE"))
    return run(x, filter_coeff, weight, bias, edge_w, src, dst, trace=trace)


def last_exec_time_ns():
    return _LAST_EXEC_NS
